# revision 16
# baseline (speedup 1.0000x reference)
"""Trainium2 Bass kernel: KV-memory retrieval (pool -> cosine kNN -> softmax gather).

Strategy (8 cores): shard the 65536-slot memory across cores (8192 keys/values
each) and the 256-image batch across cores (32 each) for pooling + output.

v2 changes vs v1 (835 us):
  - mm1 (sim) via fp16 hi/lo split: qh.kh + qh.kl + ql.kh, 3 fp16 matmuls at
    1 cycle/row vs fp32's 2-pass (measured 216 vs 429 ns per 512-row matmul).
    Probe-measured rel err 1.3e-7 == fp32, so top-k selection is unaffected.
  - key transpose via row-scale (DVE) + plain PE transpose + hi/lo split on
    the PSUM->SBUF copies, replacing the fp32 diag-matmul (4 c/r).
  - values, softmax weights, matmul2 and the broadcast output in bf16
    (host casts values to bf16 and the bf16 output back to f32; tolerance
    is 2e-2 and the selection/threshold path stays exact f32).
  - key streaming starts during the pooling phase; values prefetch during
    the top-k gap; collective staging DMAs moved to the gpsimd queue so
    bulk traffic on the sync queue never delays them.
  - AllGather staging for queries transposed on-chip so staging DMAs use
    contiguous 2KB runs (v1's unpack used 128-byte descriptors); query
    norms computed from the batch-major staging tile (free-dim reduce)
    instead of a PE ones-matmul.
  - softmax bias/normalization folded into the post-matmul2 row scale, so
    exp(sim*rinv) needs no global information.

Selection is done on raw r = q_sum . k_norm (scale-invariant per batch row);
1/||q|| enters only through the exp scale. Mean /784 cancels everywhere.
"""

import math

import numpy as np
import ml_dtypes

import concourse.bacc as bacc
import concourse.mybir as mybir
import concourse.tile as tile
from concourse.bass import ts
from concourse.bass_utils import run_bass_kernel_spmd
from concourse.masks import make_identity

F32 = mybir.dt.float32
F16 = mybir.dt.float16
BF16 = mybir.dt.bfloat16
AF = mybir.ActivationFunctionType
ALU = mybir.AluOpType

N_CORES = 8
NEG = -3.0e38


def build(B=256, C=512, HW=784, M=65536, K=32, n_cores=N_CORES, mb=512):
    """Build + bacc-compile the SPMD program. Returns nc."""
    BS = B // n_cores          # batches per core
    MS = M // n_cores          # memory slots per core
    CT = C // 128              # channel tiles (also contraction tiles)
    BT = B // 128
    BTW = 128                  # batch-tile width
    assert B % BTW == 0 and C % 128 == 0 and M % (n_cores * mb) == 0
    NMB = MS // mb             # key blocks per core
    KTPB = mb // 128           # 128-row key tiles per block
    R = math.ceil(K / 8)       # max8 rounds for exact top-K
    KPB = 16 if K == 32 else min(R * 8, mb)
    RB = KPB // 8              # rounds per block
    MT = MS // 128             # value tiles
    RG = [list(range(n_cores))]
    CC_AS = "Shared" if n_cores > 4 else "Local"
    KWIN = 4                   # kT blocks in flight
    EK = 4                     # key blocks interleaved into pooling loop
    VB = 4                     # value tiles per DMA group
    VBUF = 4                   # value groups in flight

    nc = bacc.Bacc("TRN2", target_bir_lowering=False, debug=False,
                   num_devices=n_cores)

    xs = nc.dram_tensor("xs", [BS, C, HW], F32, kind="ExternalInput").ap()
    keys = nc.dram_tensor("keys", [MS, C], F32, kind="ExternalInput").ap()
    vals = nc.dram_tensor("vals", [MS, C], BF16, kind="ExternalInput").ap()
    out = nc.dram_tensor("out", [BS, C, HW], BF16, kind="ExternalOutput").ap()

    with tile.TileContext(nc) as tc:
        with (
            tc.tile_pool(name="consts", bufs=1) as consts,
            tc.tile_pool(name="persist", bufs=1) as persist,
            tc.tile_pool(name="poolV", bufs=VBUF) as pV,
            tc.tile_pool(name="dram", bufs=1, space="DRAM") as dram,
        ):
            identity = consts.tile([128, 128], F32)
            make_identity(nc, identity)
            identity_bf = consts.tile([128, 128], BF16)
            nc.vector.tensor_copy(identity_bf, identity)
            ones_hw = consts.tile([128, HW], F32)
            nc.vector.memset(ones_hw, 1.0)

            sim = [persist.tile([BTW, MS], F32, name=f"sim{i}")
                   for i in range(BT)]
            qh = persist.tile([128, CT, B], F16, name="qh")
            ql = persist.tile([128, CT, B], F16, name="ql")
            qTl = persist.tile([128, CT, BS], F32, name="qTl")
            cand = [persist.tile([BTW, NMB * KPB], F32, name=f"cand{i}")
                    for i in range(BT)]
            rinv = [persist.tile([BTW, 1], F32, name=f"rinv{i}")
                    for i in range(BT)]
            sc = [persist.tile([BTW, 1], F32, name=f"sc{i}")
                  for i in range(BT)]
            g32 = [persist.tile([BTW, R * 8], F32, name=f"g32{i}")
                   for i in range(BT)]
            mT = [persist.tile([128, B], F32, name=f"mT{i}")
                  for i in range(CT)]
            mTmy = [persist.tile([128, BS], F32, name=f"mTmy{i}")
                    for i in range(CT)]

            CTH = CT // 2
            hw_a = 0
            for a in range(int(math.isqrt(HW)), 1, -1):
                if HW % a == 0:
                    hw_a = a
                    break

            with (
                tc.tile_pool(name="poolP", bufs=2) as pP,
                tc.tile_pool(name="poolKs", bufs=2) as pKs,
                tc.tile_pool(name="poolKn", bufs=2) as pKn,
                tc.tile_pool(name="poolKT", bufs=KWIN) as pKT,
                tc.tile_pool(name="psumA", bufs=1, space="PSUM") as psA,
            ):
                kT_tiles = []

                def prep_key_block(mbi):
                    ktb = pKs.tile([128, KTPB, C], F32, tag="ktb")
                    nc.sync.dma_start(
                        out=ktb,
                        in_=keys[mbi * mb:(mbi + 1) * mb].rearrange(
                            "(kt p) c -> p kt c", p=128))
                    kTh = pKT.tile([128, KTPB, CT, 128], F16, tag="kTh")
                    kTl = pKT.tile([128, KTPB, CT, 128], F16, tag="kTl")
                    kT_tiles.append((kTh, kTl))
                    for kt in range(KTPB):
                        ktile = ktb[:, kt]
                        ksq = pKs.tile([128, C], F32, tag="ksq")
                        ssk = pKs.tile([128, 1], F32, tag="ssk")
                        nc.scalar.activation(ksq, ktile, AF.Square,
                                             accum_out=ssk)
                        kn = pKs.tile([128, 1], F32, tag="kn")
                        nc.scalar.sqrt(kn, ssk)
                        rk = pKs.tile([128, 1], F32, tag="rk")
                        nc.vector.reciprocal(rk, kn)
                        ktn = pKn.tile([128, C], F32, tag="ktn")
                        nc.vector.tensor_scalar_mul(ktn, ktile, rk)
                        ptp = psA.tile([128, CT, 128], F32, tag="tp", bufs=2)
                        for dt in range(CT):
                            nc.tensor.matmul(
                                ptp[:, dt], lhsT=ktn[:, ts(dt, 128)],
                                rhs=identity, is_transpose=True,
                                start=True, stop=True, skip_group_check=True)
                        # contiguous copies; hi rounds to f16, lo = residual
                        nc.scalar.copy(kTh[:, kt], ptp)
                        nc.vector.tensor_sub(kTl[:, kt], ptp, kTh[:, kt])

                def mm1_block(mbi):
                    kTh, kTl = kT_tiles[mbi]
                    for bt in range(BT):
                        psim = psA.tile([BTW, mb], F32, tag="psim", bufs=3)
                        for dt in range(CT):
                            for ti, (lq, rk_) in enumerate(
                                    [(qh, kTh), (qh, kTl), (ql, kTh)]):
                                nc.tensor.matmul(
                                    psim, lhsT=lq[:, dt, ts(bt, BTW)],
                                    rhs=rk_[:, :, dt],
                                    start=(dt == 0 and ti == 0),
                                    stop=(dt == CT - 1 and ti == 2),
                                    skip_group_check=True)
                        sblk = sim[bt][:, ts(mbi, mb)]
                        if (mbi + bt) % 2 == 0:
                            nc.vector.tensor_copy(sblk, psim)
                        else:
                            nc.scalar.copy(sblk, psim)
                        cur = sblk
                        scr = pKs.tile([BTW, mb], F32, tag="scr")
                        for r in range(RB):
                            c8 = cand[bt][:, mbi * KPB + r * 8:
                                          mbi * KPB + r * 8 + 8]
                            nc.vector.max(c8, cur)
                            if r < RB - 1:
                                nc.vector.match_replace(
                                    scr, in_to_replace=c8, in_values=cur,
                                    imm_value=NEG)
                                cur = scr

                # ---------------- Phase P: pool x -> qTl ----------------
                ek_at = {2 + 6 * i: i for i in range(EK)}
                for b in range(BS):
                    xt = pP.tile([128, CT, HW], F32, tag="xt")
                    nc.sync.dma_start(
                        out=xt,
                        in_=xs[b].rearrange("(ct p) hw -> p ct hw", p=128))
                    xp = pP.tile([128, CTH, HW // hw_a], F32, tag="xp")
                    nc.vector.tensor_reduce(
                        out=xp,
                        in_=xt[:, 0:CTH].rearrange(
                            "p ct (a b) -> p ct a b", a=HW // hw_a),
                        axis=mybir.AxisListType.X, op=ALU.add)
                    xq = pP.tile([128, CTH], F32, tag="xq")
                    nc.vector.tensor_reduce(
                        out=xq, in_=xp, axis=mybir.AxisListType.X, op=ALU.add)
                    for ct in range(CTH):
                        nc.vector.tensor_copy(qTl[:, ct, b:b + 1],
                                              xq[:, ct:ct + 1])
                    for ct in range(CTH, CT):
                        xsc = pP.tile([128, HW], F32, tag="xsc")
                        nc.scalar.activation(
                            xsc, xt[:, ct], AF.Copy,
                            accum_out=qTl[:, ct, b:b + 1])
                    if b in ek_at:
                        prep_key_block(ek_at[b])

                # ---------------- AG1: gather queries (transposed) -------
                qag_in = dram.tile([BS, C], F32)
                qag_out = dram.tile([B, C], F32, addr_space=CC_AS)
                with tc.tile_pool(name="poolAG", bufs=1) as pAG:
                    pq = psA.tile([BS, C], F32, tag="pq", bufs=1)
                    for ct in range(CT):
                        nc.tensor.matmul(
                            pq[:, ts(ct, 128)], lhsT=qTl[:, ct],
                            rhs=identity, is_transpose=True,
                            start=True, stop=True, skip_group_check=True)
                    qrow = pAG.tile([BS, C], F32, tag="qrow")
                    nc.vector.tensor_copy(qrow, pq)
                    nc.gpsimd.dma_start(out=qag_in, in_=qrow)
                    nc.gpsimd.collective_compute(
                        "AllGather", ALU.bypass, replica_groups=RG,
                        ins=[qag_in.opt()], outs=[qag_out.opt()])
                    # load [B, C] (2KB rows), transpose back on PE
                    qbc = pAG.tile([128, BT, C], F32, tag="qbc")
                    nc.gpsimd.dma_start(
                        out=qbc,
                        in_=qag_out.rearrange("(bt p) c -> p bt c", p=128))
                    for bt in range(BT):
                        pqt = psA.tile([128, CT, 128], F32, tag="tp", bufs=2)
                        for ct in range(CT):
                            nc.tensor.matmul(
                                pqt[:, ct],
                                lhsT=qbc[:, bt, ts(ct, 128)],
                                rhs=identity, is_transpose=True,
                                start=True, stop=True, skip_group_check=True)
                        nc.scalar.copy(qh[:, :, ts(bt, BTW)], pqt)
                        nc.vector.tensor_sub(ql[:, :, ts(bt, BTW)], pqt,
                                             qh[:, :, ts(bt, BTW)])
                        # query norms from the batch-major tile (free reduce)
                        qsqd = pAG.tile([128, C], F32, tag="qsqd")
                        ssqb = pAG.tile([128, 1], F32, tag="ssqb")
                        nc.scalar.activation(qsqd, qbc[:, bt], AF.Square,
                                             accum_out=ssqb)
                        qn = pAG.tile([128, 1], F32, tag="qn")
                        nc.scalar.sqrt(qn, ssqb)
                        nc.vector.reciprocal(rinv[bt], qn)

                # ---------------- Phase K: keys -> sim + candidates ------
                for mbi in range(NMB):
                    if mbi >= EK:
                        prep_key_block(mbi)
                    mm1_block(mbi)

                # values prefetch: top-level pool space, dispatch right
                # after the key DMAs so they fill the AG1/top-k DMA gap
                vt_tiles = []
                for g in range(MT // VB):
                    vtb = pV.tile([128, VB, C], BF16, tag="vtb")
                    nc.sync.dma_start(
                        out=vtb,
                        in_=vals[g * VB * 128:(g + 1) * VB * 128].rearrange(
                            "(v p) c -> p v c", p=128))
                    vt_tiles.append(vtb)

            # ---------------- Phase G + W ----------
            cd_in = dram.tile([B, K], F32)
            cd_out = dram.tile([n_cores, B, K], F32, addr_space=CC_AS)
            with (
                tc.tile_pool(name="poolG", bufs=1) as pG,
                tc.tile_pool(name="poolWc", bufs=4) as pWc,
                tc.tile_pool(name="poolWs", bufs=3) as pWs,
                tc.tile_pool(name="psumW", bufs=1, space="PSUM") as psW,
            ):
                # exp chunks (need only rinv) -- the first few run during
                # the candidate AllGather window on the otherwise-idle ACT
                WL = 4  # == poolWc bufs

                wch = {}

                def emit_exp(mbi):
                    for bt in range(BT):
                        w = pWc.tile([BTW, mb], BF16, tag=f"wc{bt}")
                        wch[(mbi, bt)] = w
                        nc.scalar.activation(w, sim[bt][:, ts(mbi, mb)],
                                             AF.Exp, scale=rinv[bt])

                for mbi in range(WL):
                    emit_exp(mbi)

                # ---- Phase G: global top-K + softmax stats ----
                for bt in range(BT):
                    loc = pG.tile([BTW, R * 8], F32, tag="loc")
                    scr2 = pG.tile([BTW, NMB * KPB], F32, tag="scr2")
                    cur = cand[bt]
                    for r in range(R):
                        nc.vector.max(loc[:, r * 8:(r + 1) * 8], cur)
                        if r < R - 1:
                            nc.vector.match_replace(
                                scr2, in_to_replace=loc[:, r * 8:(r + 1) * 8],
                                in_values=cur, imm_value=NEG)
                            cur = scr2
                    nc.gpsimd.dma_start(out=cd_in[ts(bt, BTW), :],
                                        in_=loc[:, 0:K])
                nc.gpsimd.collective_compute(
                    "AllGather", ALU.bypass, replica_groups=RG,
                    ins=[cd_in.opt()], outs=[cd_out.opt()])
                for bt in range(BT):
                    gc = pG.tile([BTW, n_cores, K], F32, tag="gc")
                    nc.gpsimd.dma_start(
                        out=gc,
                        in_=cd_out[:, ts(bt, BTW), :].rearrange(
                            "r p k -> p r k"))
                    gcf = gc.rearrange("p r k -> p (r k)")
                    scr3 = pG.tile([BTW, n_cores * K], F32, tag="scr3")
                    cur = gcf
                    for r in range(R):
                        nc.vector.max(g32[bt][:, r * 8:(r + 1) * 8], cur)
                        if r < R - 1:
                            nc.vector.match_replace(
                                scr3,
                                in_to_replace=g32[bt][:, r * 8:(r + 1) * 8],
                                in_values=cur, imm_value=NEG)
                            cur = scr3
                    # stats: nb = -gmax*rinv ; Z = sum exp((g - gmax)*rinv)
                    nb = pG.tile([BTW, 1], F32, tag="nb")
                    nc.vector.tensor_mul(nb, g32[bt][:, 0:1], rinv[bt])
                    nc.vector.tensor_scalar_mul(nb, nb, -1.0)
                    ex = pG.tile([BTW, K], F32, tag="ex")
                    zz = pG.tile([BTW, 1], F32, tag="zz")
                    nc.scalar.activation(ex, g32[bt][:, 0:K], AF.Exp,
                                         bias=nb, scale=rinv[bt],
                                         accum_out=zz)
                    lnz = pG.tile([BTW, 1], F32, tag="lnz")
                    nc.scalar.activation(lnz, zz, AF.Ln)
                    b2 = pG.tile([BTW, 1], F32, tag="b2")
                    nc.vector.tensor_sub(b2, nb, lnz)
                    nc.scalar.activation(sc[bt], b2, AF.Exp)

                # ---- Phase W: mask (bf16) + matmul2, per key block ----
                # per-block pipeline; exp for block i+WL emitted at the END
                # of iteration i to keep the in-order engine queues acyclic
                pm = [psW.tile([128, B], F32, tag=f"pm{dt}",
                               name=f"pm{dt}") for dt in range(CT)]
                wT_prev = None
                for mbi in range(NMB):
                    nc.vector.scalar_tensor_tensor(
                        out=wch[(mbi, 0)], in0=sim[0][:, ts(mbi, mb)],
                        scalar=g32[0][:, K - 1:K], in1=wch[(mbi, 0)],
                        op0=ALU.is_ge, op1=ALU.mult)
                    nc.vector.scalar_tensor_tensor(
                        out=wch[(mbi, 1)], in0=sim[1][:, ts(mbi, mb)],
                        scalar=g32[1][:, K - 1:K], in1=wch[(mbi, 1)],
                        op0=ALU.is_ge, op1=ALU.mult)
                    for lt in range(KTPB):
                        mt = mbi * KTPB + lt
                        pwt = psW.tile([128, B], BF16, tag="pwt", bufs=3)
                        for bt in range(BT):
                            nc.tensor.matmul(
                                pwt[:, ts(bt, BTW)],
                                lhsT=wch[(mbi, bt)][:, ts(lt, 128)],
                                rhs=identity_bf[0:BTW, 0:BTW],
                                is_transpose=True,
                                start=True, stop=True,
                                skip_group_check=True)
                        wT = pWs.tile([128, B], BF16, tag="wT")
                        if mt % 2 == 0:
                            nc.vector.tensor_copy(wT, pwt)
                        else:
                            nc.scalar.copy(wT, pwt)
                        if mt > 0:
                            pv = mt - 1
                            vt = vt_tiles[pv // VB][:, pv % VB]
                            for dt in range(CT):
                                nc.tensor.matmul(
                                    pm[dt], lhsT=vt[:, ts(dt, 128)],
                                    rhs=wT_prev,
                                    start=(pv == 0), stop=(pv == MT - 1),
                                    skip_group_check=True)
                        wT_prev = wT
                    if mbi + WL < NMB:
                        emit_exp(mbi + WL)
                # final mm2 for the last tile
                pv = MT - 1
                vt = vt_tiles[pv // VB][:, pv % VB]
                for dt in range(CT):
                    nc.tensor.matmul(
                        pm[dt], lhsT=vt[:, ts(dt, 128)], rhs=wT_prev,
                        start=(pv == 0), stop=(pv == MT - 1),
                        skip_group_check=True)
                for dt in range(CT):
                    nc.any.tensor_copy(mT[dt], pm[dt])

            # ---------------- Phase O: reduce-scatter + broadcast out ----
            mb_dram = dram.tile([B, C], F32)
            rs_out = dram.tile([BS, C], F32)
            with (
                tc.tile_pool(name="poolO", bufs=2) as pO,
                tc.tile_pool(name="psumO", bufs=1, space="PSUM") as psO,
            ):
                for bt in range(BT):
                    pmb = psO.tile([BTW, C], F32, tag="pmb", bufs=2)
                    for dt in range(CT):
                        nc.tensor.matmul(
                            pmb[:, ts(dt, 128)], lhsT=mT[dt][:, ts(bt, BTW)],
                            rhs=identity, is_transpose=True,
                            start=True, stop=True, skip_group_check=True)
                    mrow = pO.tile([BTW, C], F32, tag="mrow")
                    # fold softmax normalization exp(-gmax*rinv - lnZ) here
                    nc.scalar.mul(mrow, pmb, sc[bt])
                    nc.gpsimd.dma_start(out=mb_dram[ts(bt, BTW), :], in_=mrow)
                nc.gpsimd.collective_compute(
                    "ReduceScatter", ALU.add, replica_groups=RG,
                    ins=[mb_dram.opt()], outs=[rs_out.opt()])
                mmy = pO.tile([BS, C], F32, tag="mmy", bufs=1)
                nc.gpsimd.dma_start(out=mmy, in_=rs_out)
                for dt in range(CT):
                    pmt = psO.tile([128, BS], F32, tag="pmt", bufs=2)
                    nc.tensor.matmul(
                        pmt, lhsT=mmy[:, ts(dt, 128)],
                        rhs=identity[0:BS, 0:BS], is_transpose=True,
                        start=True, stop=True, skip_group_check=True)
                    nc.any.tensor_copy(mTmy[dt], pmt)
                for b in range(BS):
                    ot = pO.tile([128, CT, HW], BF16, tag="ot", bufs=4)
                    for dt in range(CT):
                        col = mTmy[dt][:, b:b + 1]
                        if dt < CT // 2:
                            nc.vector.tensor_scalar_mul(ot[:, dt], ones_hw,
                                                        col)
                        else:
                            nc.scalar.mul(ot[:, dt], ones_hw, col)
                    nc.sync.dma_start(
                        out=out[b].rearrange("(ct p) hw -> p ct hw", p=128),
                        in_=ot)

    nc.compile()
    return nc


_CACHE = {}
TRACE = False
LAST_RESULT = None


def _get(shape_key):
    if shape_key not in _CACHE:
        _CACHE[shape_key] = build(*shape_key)
    return _CACHE[shape_key]


def kernel(x, keys, values, topk, **_ignored):
    K = int(np.asarray(topk))
    B, C, H, W = x.shape
    M, D = keys.shape
    HW = H * W
    nc = _get((B, C, HW, M, K, N_CORES))
    BS, MS = B // N_CORES, M // N_CORES
    x3 = np.ascontiguousarray(x.reshape(B, C, HW)).astype(np.float32,
                                                          copy=False)
    keys = np.ascontiguousarray(keys).astype(np.float32, copy=False)
    vals_bf = np.ascontiguousarray(values).astype(ml_dtypes.bfloat16)
    in_maps = [{
        "xs": x3[c * BS:(c + 1) * BS],
        "keys": keys[c * MS:(c + 1) * MS],
        "vals": vals_bf[c * MS:(c + 1) * MS],
    } for c in range(N_CORES)]
    global LAST_RESULT
    res = run_bass_kernel_spmd(nc, in_maps, core_ids=list(range(N_CORES)),
                               trace=TRACE)
    LAST_RESULT = res
    outs = [np.asarray(res.results[c]["out"]).astype(np.float32)
            for c in range(N_CORES)]
    return np.concatenate(outs, axis=0).reshape(B, C, H, W)


# revision 22
# speedup vs baseline: 1.0665x; 1.0665x over previous
"""Trainium2 Bass kernel: KV-memory retrieval (pool -> cosine kNN -> softmax gather).

Strategy (8 cores): shard the 65536-slot memory across cores (8192 keys/values
each) and the 256-image batch across cores (32 each) for pooling + output.

v2 changes vs v1 (835 us):
  - mm1 (sim) via fp16 hi/lo split: qh.kh + qh.kl + ql.kh, 3 fp16 matmuls at
    1 cycle/row vs fp32's 2-pass (measured 216 vs 429 ns per 512-row matmul).
    Probe-measured rel err 1.3e-7 == fp32, so top-k selection is unaffected.
  - key transpose via row-scale (DVE) + plain PE transpose + hi/lo split on
    the PSUM->SBUF copies, replacing the fp32 diag-matmul (4 c/r).
  - values, softmax weights, matmul2 and the broadcast output in bf16
    (host casts values to bf16 and the bf16 output back to f32; tolerance
    is 2e-2 and the selection/threshold path stays exact f32).
  - key streaming starts during the pooling phase; values prefetch during
    the top-k gap; collective staging DMAs moved to the gpsimd queue so
    bulk traffic on the sync queue never delays them.
  - AllGather staging for queries transposed on-chip so staging DMAs use
    contiguous 2KB runs (v1's unpack used 128-byte descriptors); query
    norms computed from the batch-major staging tile (free-dim reduce)
    instead of a PE ones-matmul.
  - softmax bias/normalization folded into the post-matmul2 row scale, so
    exp(sim*rinv) needs no global information.

Selection is done on raw r = q_sum . k_norm (scale-invariant per batch row);
1/||q|| enters only through the exp scale. Mean /784 cancels everywhere.
"""

import math

import numpy as np
import ml_dtypes

import concourse.bacc as bacc
import concourse.mybir as mybir
import concourse.tile as tile
from concourse.bass import ts
from concourse.bass_utils import run_bass_kernel_spmd
from concourse.masks import make_identity

F32 = mybir.dt.float32
F16 = mybir.dt.float16
BF16 = mybir.dt.bfloat16
AF = mybir.ActivationFunctionType
ALU = mybir.AluOpType

N_CORES = 8
NEG = -3.0e38


def build(B=256, C=512, HW=784, M=65536, K=32, n_cores=N_CORES, mb=512):
    """Build + bacc-compile the SPMD program. Returns nc."""
    BS = B // n_cores          # batches per core
    MS = M // n_cores          # memory slots per core
    CT = C // 128              # channel tiles (also contraction tiles)
    BT = B // 128
    BTW = 128                  # batch-tile width
    assert B % BTW == 0 and C % 128 == 0 and M % (n_cores * mb) == 0
    NMB = MS // mb             # key blocks per core
    KTPB = mb // 128           # 128-row key tiles per block
    R = math.ceil(K / 8)       # max8 rounds for exact top-K
    KPB = 16 if K == 32 else min(R * 8, mb)
    RB = KPB // 8              # rounds per block
    MT = MS // 128             # value tiles
    RG = [list(range(n_cores))]
    CC_AS = "Shared" if n_cores > 4 else "Local"
    KWIN = 4                   # kT blocks in flight
    EK = 4                     # key blocks interleaved into pooling loop
    VB = 4                     # value tiles per DMA group
    VBUF = 4                   # value groups in flight

    nc = bacc.Bacc("TRN2", target_bir_lowering=False, debug=False,
                   num_devices=n_cores)

    xs = nc.dram_tensor("xs", [BS, C, HW], F32, kind="ExternalInput").ap()
    keys = nc.dram_tensor("keys", [MS, C], F32, kind="ExternalInput").ap()
    vals = nc.dram_tensor("vals", [MS, C], BF16, kind="ExternalInput").ap()
    out = nc.dram_tensor("out", [BS, C, HW], BF16, kind="ExternalOutput").ap()

    with tile.TileContext(nc) as tc:
        with (
            tc.tile_pool(name="consts", bufs=1) as consts,
            tc.tile_pool(name="persist", bufs=1) as persist,
            tc.tile_pool(name="poolV", bufs=VBUF) as pV,
            tc.tile_pool(name="dram", bufs=1, space="DRAM") as dram,
        ):
            identity = consts.tile([128, 128], F32)
            make_identity(nc, identity)
            identity_bf = consts.tile([128, 128], BF16)
            nc.vector.tensor_copy(identity_bf, identity)
            ones_hw = consts.tile([128, HW], F32)
            nc.vector.memset(ones_hw, 1.0)

            sim = [persist.tile([BTW, MS], F32, name=f"sim{i}")
                   for i in range(BT)]
            qh = persist.tile([128, CT, B], F16, name="qh")
            ql = persist.tile([128, CT, B], F16, name="ql")
            qTl = persist.tile([128, CT, BS], F32, name="qTl")
            cand = [persist.tile([BTW, NMB * KPB], F32, name=f"cand{i}")
                    for i in range(BT)]
            rinv = [persist.tile([BTW, 1], F32, name=f"rinv{i}")
                    for i in range(BT)]
            sc = [persist.tile([BTW, 1], F32, name=f"sc{i}")
                  for i in range(BT)]
            g32 = [persist.tile([BTW, R * 8], F32, name=f"g32{i}")
                   for i in range(BT)]
            mT = [persist.tile([128, B], F32, name=f"mT{i}")
                  for i in range(CT)]
            mTmy = [persist.tile([128, BS], F32, name=f"mTmy{i}")
                    for i in range(CT)]

            CTH = CT // 2
            hw_a = 0
            for a in range(int(math.isqrt(HW)), 1, -1):
                if HW % a == 0:
                    hw_a = a
                    break

            with (
                tc.tile_pool(name="poolP", bufs=2) as pP,
                tc.tile_pool(name="poolKs", bufs=2) as pKs,
                tc.tile_pool(name="poolKn", bufs=2) as pKn,
                tc.tile_pool(name="poolKT", bufs=KWIN) as pKT,
                tc.tile_pool(name="psumA", bufs=1, space="PSUM") as psA,
            ):
                kT_tiles = []

                def prep_key_block(mbi):
                    ktb = pKs.tile([128, KTPB, C], F32, tag="ktb")
                    nc.sync.dma_start(
                        out=ktb,
                        in_=keys[mbi * mb:(mbi + 1) * mb].rearrange(
                            "(kt p) c -> p kt c", p=128))
                    kTh = pKT.tile([128, KTPB, CT, 128], F16, tag="kTh")
                    kTl = pKT.tile([128, KTPB, CT, 128], F16, tag="kTl")
                    kT_tiles.append((kTh, kTl))
                    for kt in range(KTPB):
                        ktile = ktb[:, kt]
                        ksq = pKs.tile([128, C], F32, tag="ksq")
                        ssk = pKs.tile([128, 1], F32, tag="ssk")
                        nc.scalar.activation(ksq, ktile, AF.Square,
                                             accum_out=ssk)
                        kn = pKs.tile([128, 1], F32, tag="kn")
                        nc.scalar.sqrt(kn, ssk)
                        rk = pKs.tile([128, 1], F32, tag="rk")
                        nc.vector.reciprocal(rk, kn)
                        ktn = pKn.tile([128, C], F32, tag="ktn")
                        nc.vector.tensor_scalar_mul(ktn, ktile, rk)
                        ptp = psA.tile([128, CT, 128], F32, tag="tp", bufs=2)
                        for dt in range(CT):
                            nc.tensor.matmul(
                                ptp[:, dt], lhsT=ktn[:, ts(dt, 128)],
                                rhs=identity, is_transpose=True,
                                start=True, stop=True, skip_group_check=True)
                        # contiguous copies; hi rounds to f16, lo = residual
                        nc.scalar.copy(kTh[:, kt], ptp)
                        nc.vector.tensor_sub(kTl[:, kt], ptp, kTh[:, kt])

                def mm1_block(mbi):
                    kTh, kTl = kT_tiles[mbi]
                    for bt in range(BT):
                        psim = psA.tile([BTW, mb], F32, tag="psim", bufs=3)
                        for dt in range(CT):
                            for ti, (lq, rk_) in enumerate(
                                    [(qh, kTh), (qh, kTl), (ql, kTh)]):
                                nc.tensor.matmul(
                                    psim, lhsT=lq[:, dt, ts(bt, BTW)],
                                    rhs=rk_[:, :, dt],
                                    start=(dt == 0 and ti == 0),
                                    stop=(dt == CT - 1 and ti == 2),
                                    skip_group_check=True)
                        sblk = sim[bt][:, ts(mbi, mb)]
                        if (mbi + bt) % 2 == 0:
                            nc.vector.tensor_copy(sblk, psim)
                        else:
                            nc.scalar.copy(sblk, psim)
                        cur = sblk
                        scr = pKs.tile([BTW, mb], F32, tag="scr")
                        for r in range(RB):
                            c8 = cand[bt][:, mbi * KPB + r * 8:
                                          mbi * KPB + r * 8 + 8]
                            nc.vector.max(c8, cur)
                            if r < RB - 1:
                                nc.vector.match_replace(
                                    scr, in_to_replace=c8, in_values=cur,
                                    imm_value=NEG)
                                cur = scr

                # ---------------- Phase P: pool x -> qTl ----------------
                ek_at = {2 + 6 * i: i for i in range(EK)}
                for b in range(BS):
                    xt = pP.tile([128, CT, HW], F32, tag="xt")
                    nc.sync.dma_start(
                        out=xt,
                        in_=xs[b].rearrange("(ct p) hw -> p ct hw", p=128))
                    xp = pP.tile([128, CTH, HW // hw_a], F32, tag="xp")
                    nc.vector.tensor_reduce(
                        out=xp,
                        in_=xt[:, 0:CTH].rearrange(
                            "p ct (a b) -> p ct a b", a=HW // hw_a),
                        axis=mybir.AxisListType.X, op=ALU.add)
                    xq = pP.tile([128, CTH], F32, tag="xq")
                    nc.vector.tensor_reduce(
                        out=xq, in_=xp, axis=mybir.AxisListType.X, op=ALU.add)
                    for ct in range(CTH):
                        nc.vector.tensor_copy(qTl[:, ct, b:b + 1],
                                              xq[:, ct:ct + 1])
                    for ct in range(CTH, CT):
                        xsc = pP.tile([128, HW], F32, tag="xsc")
                        nc.scalar.activation(
                            xsc, xt[:, ct], AF.Copy,
                            accum_out=qTl[:, ct, b:b + 1])
                    if b in ek_at:
                        prep_key_block(ek_at[b])

                # ---------------- AG1: gather queries (transposed) -------
                qag_in = dram.tile([BS, C], F32)
                qag_out = dram.tile([B, C], F32, addr_space=CC_AS)
                with tc.tile_pool(name="poolAG", bufs=1) as pAG:
                    pq = psA.tile([BS, C], F32, tag="pq", bufs=1)
                    for ct in range(CT):
                        nc.tensor.matmul(
                            pq[:, ts(ct, 128)], lhsT=qTl[:, ct],
                            rhs=identity, is_transpose=True,
                            start=True, stop=True, skip_group_check=True)
                    qrow = pAG.tile([BS, C], F32, tag="qrow")
                    nc.vector.tensor_copy(qrow, pq)
                    nc.gpsimd.dma_start(out=qag_in, in_=qrow)
                    nc.gpsimd.collective_compute(
                        "AllGather", ALU.bypass, replica_groups=RG,
                        ins=[qag_in.opt()], outs=[qag_out.opt()])
                    # load [B, C] (2KB rows), transpose back on PE
                    qbc = pAG.tile([128, BT, C], F32, tag="qbc")
                    nc.gpsimd.dma_start(
                        out=qbc,
                        in_=qag_out.rearrange("(bt p) c -> p bt c", p=128))
                    for bt in range(BT):
                        pqt = psA.tile([128, CT, 128], F32, tag="tp", bufs=2)
                        for ct in range(CT):
                            nc.tensor.matmul(
                                pqt[:, ct],
                                lhsT=qbc[:, bt, ts(ct, 128)],
                                rhs=identity, is_transpose=True,
                                start=True, stop=True, skip_group_check=True)
                        nc.scalar.copy(qh[:, :, ts(bt, BTW)], pqt)
                        nc.vector.tensor_sub(ql[:, :, ts(bt, BTW)], pqt,
                                             qh[:, :, ts(bt, BTW)])
                        # query norms from the batch-major tile (free reduce)
                        qsqd = pAG.tile([128, C], F32, tag="qsqd")
                        ssqb = pAG.tile([128, 1], F32, tag="ssqb")
                        nc.scalar.activation(qsqd, qbc[:, bt], AF.Square,
                                             accum_out=ssqb)
                        qn = pAG.tile([128, 1], F32, tag="qn")
                        nc.scalar.sqrt(qn, ssqb)
                        nc.vector.reciprocal(rinv[bt], qn)

                # ---------------- Phase K: keys -> sim + candidates ------
                # candidate AllGather in two halves: the first fires while
                # mm1 still runs on the second half of the key blocks
                NMBH = NMB // 2
                HC = NMBH * KPB
                cdh_in = [dram.tile([B, K], F32, name=f"cdh_in{h}")
                          for h in range(2)]
                cdh_out = [dram.tile([n_cores, B, K], F32, addr_space=CC_AS,
                                     name=f"cdh_out{h}") for h in range(2)]

                def half_topk(h):
                    for bt in range(BT):
                        loc = pKs.tile([BTW, R * 8], F32, tag="loch")
                        scrh = pKs.tile([BTW, HC], F32, tag="scrh")
                        cur = cand[bt][:, h * HC:(h + 1) * HC]
                        for r in range(R):
                            nc.vector.max(loc[:, r * 8:(r + 1) * 8], cur)
                            if r < R - 1:
                                nc.vector.match_replace(
                                    scrh,
                                    in_to_replace=loc[:, r * 8:(r + 1) * 8],
                                    in_values=cur, imm_value=NEG)
                                cur = scrh
                        nc.gpsimd.dma_start(out=cdh_in[h][ts(bt, BTW), :],
                                            in_=loc[:, 0:K])
                    nc.gpsimd.collective_compute(
                        "AllGather", ALU.bypass, replica_groups=RG,
                        ins=[cdh_in[h].opt()], outs=[cdh_out[h].opt()])

                for mbi in range(NMB):
                    if mbi >= EK:
                        prep_key_block(mbi)
                    mm1_block(mbi)
                    if mbi == NMBH - 1:
                        half_topk(0)
                half_topk(1)

                # values prefetch: top-level pool space. The first dma_start
                # is gated on the AllGather output via a dummy staging DMA --
                # value transfers running concurrently with the collective
                # stretched its barrier from 7.5us to 80us.
                vgate = dram.tile([1, 1], F32)
                nc.sync.dma_start(out=vgate, in_=rinv[0][0:1, 0:1])
                vt_tiles = []
                for g in range(MT // VB):
                    vtb = pV.tile([128, VB, C], BF16, tag="vtb")
                    nc.sync.dma_start(
                        out=vtb,
                        in_=vals[g * VB * 128:(g + 1) * VB * 128].rearrange(
                            "(v p) c -> p v c", p=128))
                    vt_tiles.append(vtb)

            # ---------------- Phase G + W ----------
            with (
                tc.tile_pool(name="poolG", bufs=1) as pG,
                tc.tile_pool(name="poolWc", bufs=4) as pWc,
                tc.tile_pool(name="poolWs", bufs=3) as pWs,
                tc.tile_pool(name="psumW", bufs=1, space="PSUM") as psW,
            ):
                # exp chunks (need only rinv) -- the first few run during
                # the candidate AllGather window on the otherwise-idle ACT
                WL = 4  # == poolWc bufs

                wch = {}

                def emit_exp(mbi):
                    for bt in range(BT):
                        w = pWc.tile([BTW, mb], BF16, tag=f"wc{bt}")
                        wch[(mbi, bt)] = w
                        nc.scalar.activation(w, sim[bt][:, ts(mbi, mb)],
                                             AF.Exp, scale=rinv[bt])

                for mbi in range(WL):
                    emit_exp(mbi)

                # ---- Phase G: merge gathered half-candidates + stats ----
                for bt in range(BT):
                    gc = pG.tile([BTW, 2, n_cores, K], F32, tag="gc")
                    for h in range(2):
                        nc.gpsimd.dma_start(
                            out=gc[:, h],
                            in_=cdh_out[h][:, ts(bt, BTW), :].rearrange(
                                "r p k -> p r k"))
                    gcf = gc.rearrange("p h r k -> p (h r k)")
                    scr3 = pG.tile([BTW, 2 * n_cores * K], F32, tag="scr3")
                    cur = gcf
                    for r in range(R):
                        nc.vector.max(g32[bt][:, r * 8:(r + 1) * 8], cur)
                        if r < R - 1:
                            nc.vector.match_replace(
                                scr3,
                                in_to_replace=g32[bt][:, r * 8:(r + 1) * 8],
                                in_values=cur, imm_value=NEG)
                            cur = scr3
                    # stats: nb = -gmax*rinv ; Z = sum exp((g - gmax)*rinv)
                    nb = pG.tile([BTW, 1], F32, tag="nb")
                    nc.vector.tensor_mul(nb, g32[bt][:, 0:1], rinv[bt])
                    nc.vector.tensor_scalar_mul(nb, nb, -1.0)
                    ex = pG.tile([BTW, K], F32, tag="ex")
                    zz = pG.tile([BTW, 1], F32, tag="zz")
                    nc.scalar.activation(ex, g32[bt][:, 0:K], AF.Exp,
                                         bias=nb, scale=rinv[bt],
                                         accum_out=zz)
                    lnz = pG.tile([BTW, 1], F32, tag="lnz")
                    nc.scalar.activation(lnz, zz, AF.Ln)
                    b2 = pG.tile([BTW, 1], F32, tag="b2")
                    nc.vector.tensor_sub(b2, nb, lnz)
                    nc.scalar.activation(sc[bt], b2, AF.Exp)

                # ---- Phase W: mask (bf16) + matmul2, per key block ----
                # per-block pipeline; exp for block i+WL emitted at the END
                # of iteration i to keep the in-order engine queues acyclic
                pm = [psW.tile([128, B], F32, tag=f"pm{dt}",
                               name=f"pm{dt}") for dt in range(CT)]
                wT_prev = None
                for mbi in range(NMB):
                    nc.vector.scalar_tensor_tensor(
                        out=wch[(mbi, 0)], in0=sim[0][:, ts(mbi, mb)],
                        scalar=g32[0][:, K - 1:K], in1=wch[(mbi, 0)],
                        op0=ALU.is_ge, op1=ALU.mult)
                    nc.vector.scalar_tensor_tensor(
                        out=wch[(mbi, 1)], in0=sim[1][:, ts(mbi, mb)],
                        scalar=g32[1][:, K - 1:K], in1=wch[(mbi, 1)],
                        op0=ALU.is_ge, op1=ALU.mult)
                    for lt in range(KTPB):
                        mt = mbi * KTPB + lt
                        pwt = psW.tile([128, B], BF16, tag="pwt", bufs=3)
                        for bt in range(BT):
                            nc.tensor.matmul(
                                pwt[:, ts(bt, BTW)],
                                lhsT=wch[(mbi, bt)][:, ts(lt, 128)],
                                rhs=identity_bf[0:BTW, 0:BTW],
                                is_transpose=True,
                                start=True, stop=True,
                                skip_group_check=True)
                        wT = pWs.tile([128, B], BF16, tag="wT")
                        if mt % 2 == 0:
                            nc.vector.tensor_copy(wT, pwt)
                        else:
                            nc.scalar.copy(wT, pwt)
                        if mt > 0:
                            pv = mt - 1
                            vt = vt_tiles[pv // VB][:, pv % VB]
                            for dt in range(CT):
                                nc.tensor.matmul(
                                    pm[dt], lhsT=vt[:, ts(dt, 128)],
                                    rhs=wT_prev,
                                    start=(pv == 0), stop=(pv == MT - 1),
                                    skip_group_check=True)
                        wT_prev = wT
                    if mbi + WL < NMB:
                        emit_exp(mbi + WL)
                # final mm2 for the last tile
                pv = MT - 1
                vt = vt_tiles[pv // VB][:, pv % VB]
                for dt in range(CT):
                    nc.tensor.matmul(
                        pm[dt], lhsT=vt[:, ts(dt, 128)], rhs=wT_prev,
                        start=(pv == 0), stop=(pv == MT - 1),
                        skip_group_check=True)
                for dt in range(CT):
                    nc.any.tensor_copy(mT[dt], pm[dt])

            # ---------------- Phase O: reduce-scatter + broadcast out ----
            mb_dram = dram.tile([B, C], F32)
            rs_out = dram.tile([BS, C], F32)
            with (
                tc.tile_pool(name="poolO", bufs=2) as pO,
                tc.tile_pool(name="psumO", bufs=1, space="PSUM") as psO,
            ):
                for bt in range(BT):
                    pmb = psO.tile([BTW, C], F32, tag="pmb", bufs=2)
                    for dt in range(CT):
                        nc.tensor.matmul(
                            pmb[:, ts(dt, 128)], lhsT=mT[dt][:, ts(bt, BTW)],
                            rhs=identity, is_transpose=True,
                            start=True, stop=True, skip_group_check=True)
                    mrow = pO.tile([BTW, C], F32, tag="mrow")
                    # fold softmax normalization exp(-gmax*rinv - lnZ) here
                    nc.scalar.mul(mrow, pmb, sc[bt])
                    nc.gpsimd.dma_start(out=mb_dram[ts(bt, BTW), :], in_=mrow)
                nc.gpsimd.collective_compute(
                    "ReduceScatter", ALU.add, replica_groups=RG,
                    ins=[mb_dram.opt()], outs=[rs_out.opt()])
                mmy = pO.tile([BS, C], F32, tag="mmy", bufs=1)
                nc.gpsimd.dma_start(out=mmy, in_=rs_out)
                for dt in range(CT):
                    pmt = psO.tile([128, BS], F32, tag="pmt", bufs=2)
                    nc.tensor.matmul(
                        pmt, lhsT=mmy[:, ts(dt, 128)],
                        rhs=identity[0:BS, 0:BS], is_transpose=True,
                        start=True, stop=True, skip_group_check=True)
                    nc.any.tensor_copy(mTmy[dt], pmt)
                for b in range(BS):
                    ot = pO.tile([128, CT, HW], BF16, tag="ot", bufs=4)
                    for dt in range(CT):
                        col = mTmy[dt][:, b:b + 1]
                        if dt < CT // 2:
                            nc.vector.tensor_scalar_mul(ot[:, dt], ones_hw,
                                                        col)
                        else:
                            nc.scalar.mul(ot[:, dt], ones_hw, col)
                    nc.sync.dma_start(
                        out=out[b].rearrange("(ct p) hw -> p ct hw", p=128),
                        in_=ot)

    nc.compile()
    return nc


_CACHE = {}
TRACE = False
LAST_RESULT = None


def _get(shape_key):
    if shape_key not in _CACHE:
        _CACHE[shape_key] = build(*shape_key)
    return _CACHE[shape_key]


def kernel(x, keys, values, topk, **_ignored):
    K = int(np.asarray(topk))
    B, C, H, W = x.shape
    M, D = keys.shape
    HW = H * W
    nc = _get((B, C, HW, M, K, N_CORES))
    BS, MS = B // N_CORES, M // N_CORES
    x3 = np.ascontiguousarray(x.reshape(B, C, HW)).astype(np.float32,
                                                          copy=False)
    keys = np.ascontiguousarray(keys).astype(np.float32, copy=False)
    vals_bf = np.ascontiguousarray(values).astype(ml_dtypes.bfloat16)
    in_maps = [{
        "xs": x3[c * BS:(c + 1) * BS],
        "keys": keys[c * MS:(c + 1) * MS],
        "vals": vals_bf[c * MS:(c + 1) * MS],
    } for c in range(N_CORES)]
    global LAST_RESULT
    res = run_bass_kernel_spmd(nc, in_maps, core_ids=list(range(N_CORES)),
                               trace=TRACE)
    LAST_RESULT = res
    outs = [np.asarray(res.results[c]["out"]).astype(np.float32)
            for c in range(N_CORES)]
    return np.concatenate(outs, axis=0).reshape(B, C, H, W)
